# revision 25
# baseline (speedup 1.0000x reference)
"""Single-step LSTM cell (NaiveLayerLSTM, INPUT_SZ=HIDDEN_SZ=4096) on 8 trn2
NeuronCores.

Sharding (tensor-parallel, per the sharding hint): core c owns hidden columns
[c*512, (c+1)*512) of every gate's weight matrix; x_t/h_t are replicated; each
core computes its 512-wide slice of the i/f/g/o gates and the c/h update
locally; the host concatenates the 8 h_new slices.  Single step, so no
collectives.

Numerics / precision scheme (per 128-row contraction chunk kk):
    x = xh + xl/2^11   (fp16 hi + fp16 lo-scaled-by-2^11, split on host)
    W = Whi + Wlo      (fp16 hi + fp8e3m4 lo, Wlo prescaled by 2^a)
    x@W ~= xh@Whi + 2^-11*(xl@Whi) + 2^-(a+b)*(xh*2^b)@(Wlo*2^a)
with all accumulation in fp32 PSUM:
  - one M=2 fp16 matmul with lhsT=[xh,xl] computes xh@Whi and xl@Whi in a
    single 512-cycle pass (PSUM rows 0/1),
  - one fp8 (e3m4) matmul accumulates the lo-correction into PSUM row 32
    (PSUM matmul base partitions must be 0/32/64),
  - biases enter PSUM row 0 via K=1 matmuls against a constant 1.0 (bf16
    hi+lo pair),
  - a tiny fp32 K=33 matmul against [1, 2^-11, 0...0, 2^-(a+b)] reduces the
    rows (cross-partition sums are impossible on DVE/ACT, trivial on PE).
fp16 values in the subnormal range are flushed to zero on the host (the lo
terms absorb them), so host math matches the PE bit-for-bit regardless of
its subnormal handling; the 2^11 scale on xl keeps xl itself out of the
subnormal range.  Measured end-to-end error vs the fp32 reference: ~9e-6
absolute on an output of scale 0.62.

Why this shape: the kernel is at the HBM/PE "ridge" — 24 MiB of weight DMA
per core (~61 us at ~395 GB/s) vs ~58 us of PE streaming (2 passes of
N=512 per chunk per gate at 1 col/cycle).  fp32 matmuls would run at 1/4
rate (232 us) and pure-fp32 DMA would be 32 MiB; the hi/lo split keeps fp32
accuracy class at bf16+fp8 byte volume and bf16 PE rate.

If h_t is all zeros (the module default initial state) the h_t@W_h* half of
the contraction is skipped entirely (checked on the actual data at runtime,
so the kernel stays correct for any input).

A short burst of warm-up matmuls on memset data runs during the ~9 us DMA
startup window so the PE's HAM clock-gate is at 8/8 (2.4 GHz) when the real
matmuls arrive.
"""

import numpy as np
import ml_dtypes

import concourse.bass as bass
import concourse.tile as tile
from concourse import bacc, mybir
from concourse.bass_utils import run_bass_kernel_spmd

BF16 = ml_dtypes.bfloat16
F8 = ml_dtypes.float8_e3m4  # matches mybir.dt.float8e3
FP16_MIN_NORMAL = 2.0 ** -14
XL_SHIFT = 2.0 ** 11
P = 128
H = 4096
NCORES = 8
HS = H // NCORES  # 512 per-core hidden slice
KX = H // P       # 32 contraction chunks for the x half
SLABK = 8         # kk-chunks per weight DMA slab
W_BUFS = 4
N_WARMUP = 18

_GATES_X = ["W_ii", "W_if", "W_ig", "W_io"]
_GATES_H = ["W_hi", "W_hf", "W_hg", "W_ho"]
_BIAS_X = ["b_ii", "b_if", "b_ig", "b_io"]
_BIAS_H = ["b_hi", "b_hf", "b_hg", "b_ho"]

_program_cache: dict = {}


def _build_program(n_kk: int):
    nc = bacc.Bacc(
        "TRN2",
        target_bir_lowering=False,
        debug=False,
        enable_asserts=False,
        num_devices=NCORES,
    )
    f32 = mybir.dt.float32
    bf16 = mybir.dt.bfloat16
    f16 = mybir.dt.float16
    f8 = mybir.dt.float8e3

    # merged weight stream: per (g,kk) block of 1536 B per partition row =
    # [whi f16 1024 B | wlo8 f8 512 B] — one DMA stream with 12 KiB
    # contiguous lines (bigger packets -> ~line-rate HBM)
    u8 = mybir.dt.uint8
    wmix_dram = nc.dram_tensor("wmix", [P, n_kk * 4 * 1536], u8, kind="ExternalInput")
    lhs_dram = nc.dram_tensor("lhs", [P, 2 * n_kk], f16, kind="ExternalInput")
    lhs8_dram = nc.dram_tensor("lhs8", [P, n_kk], f8, kind="ExternalInput")
    bias_dram = nc.dram_tensor("bias", [1, 4096], bf16, kind="ExternalInput")
    one_dram = nc.dram_tensor("one", [1, 1], bf16, kind="ExternalInput")
    red_dram = nc.dram_tensor("redvec", [33, 1], f32, kind="ExternalInput")
    ct_dram = nc.dram_tensor("ct", [1, HS], f32, kind="ExternalInput")
    out_dram = nc.dram_tensor("h_out", [1, HS], f32, kind="ExternalOutput")

    n_slabs = n_kk // SLABK
    slab_cols = SLABK * 1536

    with tile.TileContext(nc) as tc:
        with (
            tc.tile_pool(name="const", bufs=1) as const_pool,
            tc.tile_pool(name="wpool", bufs=W_BUFS) as w_pool,
            tc.tile_pool(name="psum", bufs=1, space=bass.MemorySpace.PSUM) as psum_pool,
            tc.tile_pool(name="epi", bufs=1) as epi_pool,
        ):
            # --- PE warm-up on memset data (no DMA dependency) ---
            wz = const_pool.tile([P, 512], bf16, tag="wz")
            nc.vector.memset(wz[:, :], 0.0)
            psumB = [
                psum_pool.tile([1, HS], f32, tag=f"pb{g}", name=f"psumB{g}")
                for g in range(4)
            ]
            for i in range(N_WARMUP):
                nc.tensor.matmul(
                    psumB[3][0:1, :], wz[:, 0:1], wz[:, :], start=True, stop=True
                )

            # --- constants (ACT ring, ahead of the wlo slabs) ---
            lhs_sb = const_pool.tile([P, 2 * n_kk], f16, tag="lhs")
            lhs8_sb = const_pool.tile([P, n_kk], f8, tag="lhs8")
            bias_sb = const_pool.tile([1, 4096], bf16, tag="bias")
            one_sb = const_pool.tile([1, 1], bf16, tag="one")
            red_sb = const_pool.tile([33, 1], f32, tag="red")
            ct_sb = const_pool.tile([1, HS], f32, tag="ct")
            nc.scalar.dma_start(out=lhs_sb[:, :], in_=lhs_dram[:, :])
            nc.scalar.dma_start(out=lhs8_sb[:, :], in_=lhs8_dram[:, :])
            nc.scalar.dma_start(out=bias_sb[:, :], in_=bias_dram[:, :])
            nc.scalar.dma_start(out=one_sb[:, :], in_=one_dram[:, :])
            nc.scalar.dma_start(out=red_sb[:, :], in_=red_dram[:, :])
            nc.scalar.dma_start(out=ct_sb[:, :], in_=ct_dram[:, :])

            # [33, 512]: rows 0-1 = M=2 bf16 accum, row 32 = fp8 accum (PSUM
            # base partitions must be 0/32/64), rows 2-31 zeroed and weighted
            # 0 in the reduce.
            psumA = [
                psum_pool.tile([33, HS], f32, tag=f"pa{g}", name=f"psumA{g}")
                for g in range(4)
            ]

            # --- weight stream + matmuls, gate-major ---
            for g in range(4):
                for s in range(n_slabs):
                    col0 = (g * n_kk + s * SLABK) * 1536
                    wt = w_pool.tile([P, slab_cols], u8, tag="w", name=f"w{g}_{s}")
                    nc.sync.dma_start(out=wt[:, :], in_=wmix_dram[:, col0:col0 + slab_cols])
                    for j in range(SLABK):
                        kk = s * SLABK + j
                        first = kk == 0
                        last = kk == n_kk - 1
                        whi_rhs = wt[:, j * 1536:j * 1536 + 1024].bitcast(f16)
                        wlo_rhs = wt[:, j * 1536 + 1024:(j + 1) * 1536].bitcast(f8)
                        if first:
                            # open the accumulation group: zero rows 0-32
                            nc.tensor.matmul(
                                psumA[g][0:33, :], wz[:, 0:33], wz[:, :],
                                start=True, stop=False,
                            )
                        nc.tensor.matmul(
                            psumA[g][0:2, :],
                            lhs_sb[:, 2 * kk:2 * kk + 2],
                            whi_rhs,
                            start=False,
                            stop=last,
                        )
                        if first:
                            # biases: K=1 matmuls into row 0 (hi + lo)
                            nc.tensor.matmul(
                                psumA[g][0:1, :],
                                one_sb[0:1, 0:1],
                                bias_sb[0:1, (g * 2) * 512:(g * 2 + 1) * 512],
                                start=False, stop=False,
                            )
                            nc.tensor.matmul(
                                psumA[g][0:1, :],
                                one_sb[0:1, 0:1],
                                bias_sb[0:1, (g * 2 + 1) * 512:(g * 2 + 2) * 512],
                                start=False, stop=False,
                            )
                        nc.tensor.matmul(
                            psumA[g][32:33, :],
                            lhs8_sb[:, kk:kk + 1],
                            wlo_rhs,
                            start=False,
                            stop=last,
                        )

            # --- per-gate: copy 3 PSUM rows to SBUF, fp32 K=3 reduce matmul
            #     against [1, 1, descale], then the gate activation ---
            act = []
            for g in range(4):
                rows = epi_pool.tile([33, HS], f32, tag=f"rows{g}", name=f"rows{g}")
                nc.scalar.copy(rows[0:33, :], psumA[g][0:33, :])
                nc.tensor.matmul(
                    psumB[g][0:1, :], red_sb[0:33, 0:1], rows[0:33, :],
                    start=True, stop=True,
                )
                a = epi_pool.tile([1, HS], f32, tag=f"act{g}", name=f"act{g}")
                func = (
                    mybir.ActivationFunctionType.Tanh
                    if g == 2
                    else mybir.ActivationFunctionType.Sigmoid
                )
                nc.scalar.activation(a[0:1, :], psumB[g][0:1, :], func)
                act.append(a)

            i_t, f_t, g_t, o_t = act
            ig = epi_pool.tile([1, HS], f32, tag="ig")
            fc = epi_pool.tile([1, HS], f32, tag="fc")
            cn = epi_pool.tile([1, HS], f32, tag="cn")
            tn = epi_pool.tile([1, HS], f32, tag="tn")
            hh = epi_pool.tile([1, HS], f32, tag="hh")
            nc.vector.tensor_mul(ig[0:1, :], i_t[0:1, :], g_t[0:1, :])
            nc.vector.tensor_mul(fc[0:1, :], f_t[0:1, :], ct_sb[0:1, :])
            nc.vector.tensor_add(cn[0:1, :], ig[0:1, :], fc[0:1, :])
            nc.scalar.activation(tn[0:1, :], cn[0:1, :], mybir.ActivationFunctionType.Tanh)
            nc.vector.tensor_mul(hh[0:1, :], o_t[0:1, :], tn[0:1, :])
            nc.sync.dma_start(out=out_dram[:, :], in_=hh[0:1, :])

    nc.compile()
    return nc


def _split_hi_lo_f32(a: np.ndarray):
    """fp32 -> (bf16-as-f32 hi, f32 residual lo)."""
    a = np.ascontiguousarray(a, dtype=np.float32)
    hi = a.astype(BF16)
    return hi, a - hi.astype(np.float32)


def _split16(a: np.ndarray):
    """fp32 -> (fp16 hi with subnormals flushed to 0, f32 residual lo)."""
    a = np.ascontiguousarray(a, dtype=np.float32)
    hi = a.astype(np.float16)
    hi = np.where(np.abs(hi) < FP16_MIN_NORMAL, np.float16(0), hi)
    return hi, a - hi.astype(np.float32)


def run(inputs: dict, trace: bool = False):
    """Returns (h_new [4096] f32, exec_time_ns or None)."""
    if trace:
        _ensure_ntff_hook()
    inputs = {k: np.asarray(v) for k, v in inputs.items()}
    x = inputs["x_t"].astype(np.float32)
    h = inputs["h_t"].astype(np.float32)
    c = inputs["c_t"].astype(np.float32)

    h_zero = not np.any(h)
    n_kk = KX if h_zero else 2 * KX

    if n_kk not in _program_cache:
        _program_cache[n_kk] = _build_program(n_kk)
    nc = _program_cache[n_kk]

    f8max = float(ml_dtypes.finfo(F8).max)

    # lhs vector: x (and h when nonzero), fp16 hi + fp16 lo*2^11 per chunk
    vec = x if h_zero else np.concatenate([x, h]).astype(np.float32)
    vhi, vlo_f = _split16(vec)
    vhi_f = vhi.astype(np.float32)
    vlo = (vlo_f * XL_SHIFT).astype(np.float16)
    vlo = np.where(np.abs(vlo) < FP16_MIN_NORMAL, np.float16(0), vlo)
    lhs = np.ascontiguousarray(
        np.stack(
            [vhi.reshape(n_kk, P), vlo.reshape(n_kk, P)], axis=2
        ).transpose(1, 0, 2).reshape(P, 2 * n_kk)
    )
    # fp8 copy of the hi vector, scaled by 2^b
    vmax = np.abs(vhi_f).max()
    b_exp = float(np.floor(np.log2((f8max / 2) / max(vmax, 1e-30))))
    lhs8 = np.ascontiguousarray(
        (vhi_f * 2.0**b_exp).astype(F8).reshape(n_kk, P).T
    )

    # weight split (full matrices once; slice per core below)
    whis, wlos = [], []
    wlo_max = 0.0
    for g in range(4):
        wx = np.asarray(inputs[_GATES_X[g]], dtype=np.float32)
        if not h_zero:
            wx = np.concatenate(
                [wx, np.asarray(inputs[_GATES_H[g]], dtype=np.float32)], axis=0
            )
        hi, lo_f = _split16(wx)
        wlo_max = max(wlo_max, float(np.abs(lo_f).max()))
        whis.append(hi)
        wlos.append(lo_f)
    a_exp = float(np.floor(np.log2((f8max / 2) / max(wlo_max, 1e-30))))
    descale = np.float32(2.0 ** (-(a_exp + b_exp)))
    redvec = np.zeros((33, 1), dtype=np.float32)
    redvec[0, 0] = 1.0
    redvec[1, 0] = np.float32(1.0 / XL_SHIFT)
    redvec[32, 0] = descale
    one = np.ones((1, 1), dtype=BF16)

    in_maps = []
    for core in range(NCORES):
        sl = slice(core * HS, (core + 1) * HS)
        wmix_blocks = []
        for g in range(4):
            hi = np.ascontiguousarray(whis[g][:, sl])  # [n_kk*128, 512] fp16
            lo8 = (wlos[g][:, sl] * 2.0**a_exp).astype(F8)
            # per row: [1024 B of fp16 | 512 B of fp8]
            mix = np.concatenate(
                [hi.view(np.uint8).reshape(n_kk * P, 1024),
                 lo8.view(np.uint8).reshape(n_kk * P, 512)],
                axis=1,
            )  # [n_kk*128, 1536] u8
            wmix_blocks.append(
                mix.reshape(n_kk, P, 1536).transpose(1, 0, 2).reshape(P, n_kk * 1536)
            )
        bias = np.empty((1, 4096), dtype=BF16)
        for g in range(4):
            bb = (
                np.asarray(inputs[_BIAS_X[g]], dtype=np.float32)
                + np.asarray(inputs[_BIAS_H[g]], dtype=np.float32)
            )[sl]
            bhi, blo_f = _split_hi_lo_f32(bb)
            bias[0, (g * 2) * 512:(g * 2 + 1) * 512] = bhi
            bias[0, (g * 2 + 1) * 512:(g * 2 + 2) * 512] = blo_f.astype(BF16)
        in_maps.append(
            {
                "wmix": np.ascontiguousarray(np.concatenate(wmix_blocks, axis=1)),
                "lhs": lhs,
                "lhs8": lhs8,
                "bias": bias,
                "one": one,
                "redvec": redvec,
                "ct": np.ascontiguousarray(c[sl]).reshape(1, HS),
            }
        )

    res = run_bass_kernel_spmd(
        nc, in_maps, core_ids=list(range(NCORES)), trace=trace
    )
    out = np.concatenate(
        [np.asarray(res.results[core]["h_out"][0], dtype=np.float32)
         for core in range(NCORES)]
    )
    return out, res.exec_time_ns


def _ensure_ntff_hook():
    """Register the axon NTFF profile hook if boot-time registration was
    skipped (antenv.axon_hooks missing from the agent image).  Test-only."""
    import os
    import sys
    import types

    try:
        from antenv.axon_hooks import get_axon_ntff_profile_hook  # noqa: F401
        return
    except ImportError:
        pass
    mod = types.ModuleType("antenv.axon_hooks")
    mod._hook = None

    def set_axon_ntff_profile_hook(h):
        mod._hook = h

    def get_axon_ntff_profile_hook():
        return mod._hook

    mod.set_axon_ntff_profile_hook = set_axon_ntff_profile_hook
    mod.get_axon_ntff_profile_hook = get_axon_ntff_profile_hook
    sys.modules["antenv.axon_hooks"] = mod
    try:
        import antenv

        antenv.axon_hooks = mod
    except ImportError:
        pass
    try:
        from trn_agent_boot.trn_boot import _ntff_profile_via_ctypes

        for so in ("/opt/axon/libaxon_pjrt.so", "/root/.axon_site/libaxon_pjrt.so"):
            if os.path.exists(so):
                mod._hook = _ntff_profile_via_ctypes(so)
                break
    except Exception as e:  # degrade to no-trace
        print(f"ntff hook unavailable: {e!r}", file=sys.stderr)


def kernel(**inputs) -> np.ndarray:
    out, _ = run(inputs)
    return out


# revision 41
# speedup vs baseline: 1.4172x; 1.4172x over previous
"""Single-step LSTM cell (NaiveLayerLSTM, INPUT_SZ=HIDDEN_SZ=4096) on 8 trn2
NeuronCores.

Sharding (tensor-parallel, per the sharding hint): core c owns hidden columns
[c*512, (c+1)*512) of every gate's weight matrix; x_t/h_t are replicated; each
core computes its 512-wide slice of the i/f/g/o gates and the c/h update
locally; the host concatenates the 8 h_new slices.  Single step, so no
collectives.

Numerics / precision scheme (per 128-row contraction chunk kk):
    x = xh + xl/2^11   (fp16 hi + fp16 lo-scaled-by-2^11, split on host)
    W = Whi [+ Wlo]    (fp16 hi [+ fp8e3m4 lo when USE_FP8, prescaled 2^a])
    x@W ~= xh@Whi + 2^-11*(xl@Whi) [+ 2^-(a+b)*(xh*2^b)@(Wlo*2^a)]
with all accumulation in fp32 PSUM:
  - one M=2 fp16 matmul with lhsT=[xh,xl] computes xh@Whi and xl@Whi in a
    single 512-cycle pass (PSUM rows 0/1),
  - (USE_FP8) one fp8 e3m4 matmul accumulates the lo-correction into PSUM
    row 32 (PSUM matmul base partitions must be 0/32/64),
  - biases enter PSUM row 0 via K=1 matmuls against a constant 1.0 (bf16
    hi+lo pair),
  - a tiny fp32r K=33 matmul against [1, 2^-11, 0...0, descale] reduces the
    rows (cross-partition sums are impossible on DVE/ACT, trivial on PE;
    the weights are powers of two so fp32r's reduced multiply is exact).
fp16 values in the subnormal range are flushed to zero on the host (the lo
terms absorb them), so host math matches the PE bit-for-bit regardless of
its subnormal handling; the 2^11 scale on xl keeps xl itself out of the
subnormal range.  Measured end-to-end error vs the fp32 reference:
~3.5e-4 absolute (fp16-only default) / ~9e-6 (USE_FP8) on an output of
scale 0.62.

Why this shape: the kernel is HBM-bound — 16 MiB of weight DMA per core
streams at ~398 GB/s (measured, = per-core HBM share) in one continuous
single-ring stream of 2 MiB slabs with 16 KiB per-partition lines; the PE
consumes each slab behind the DMA (1 pass of N=512 per chunk per gate at
1 col/cycle).  Native fp32 matmuls would run at 1/4 rate and fp32 DMA
would be 32 MiB; the fp16 hi/lo split of x keeps the x-side error at
~2^-22 so the only error is the fp16 quantization of W.

If h_t is all zeros (the module default initial state) the h_t@W_h* half of
the contraction is skipped entirely (checked on the actual data at runtime,
so the kernel stays correct for any input).
"""

import numpy as np
import ml_dtypes

import concourse.bass as bass
import concourse.tile as tile
from concourse import bacc, mybir
from concourse.bass_utils import run_bass_kernel_spmd

BF16 = ml_dtypes.bfloat16
F8 = ml_dtypes.float8_e3m4  # matches mybir.dt.float8e3
FP16_MIN_NORMAL = 2.0 ** -14
XL_SHIFT = 2.0 ** 11
P = 128
H = 4096
NCORES = 8
HS = H // NCORES  # 512 per-core hidden slice
KX = H // P       # 32 contraction chunks for the x half
W_BUFS = 6
# PE warm-up matmuls: only useful when the PE is the critical resource from
# the first real matmul.  In the DMA-bound stream the HAM ramp hides inside
# PE slack, and warm-ups DELAY the real stream (head-of-line on the PE FIFO)
# — measured +7.7 us of PE lag.  Keep 0.
N_WARMUP = 0

# True: +fp8e3m4 lo-correction of the fp16 weights (24 MiB/core DMA, ~9e-6
# abs err, ~88 us).  False: fp16 weights only (16 MiB/core DMA, ~3.5e-4 abs
# err, ~70 us).  Both are far inside bf16-class tolerance (~4e-3); default
# to the faster one.
USE_FP8 = False
# bytes per (gate, chunk) block per partition row in the merged weight stream
_BLK = 1536 if USE_FP8 else 1024
# kk-chunks per weight DMA slab: keep partition lines >= 12 KiB so DMA
# packets stay large (small packets measured ~15% below line rate)
SLABK = 8 if USE_FP8 else 16

_GATES_X = ["W_ii", "W_if", "W_ig", "W_io"]
_GATES_H = ["W_hi", "W_hf", "W_hg", "W_ho"]
_BIAS_X = ["b_ii", "b_if", "b_ig", "b_io"]
_BIAS_H = ["b_hi", "b_hf", "b_hg", "b_ho"]

_program_cache: dict = {}


def _build_program(n_kk: int):
    nc = bacc.Bacc(
        "TRN2",
        target_bir_lowering=False,
        debug=False,
        enable_asserts=False,
        num_devices=NCORES,
    )
    f32 = mybir.dt.float32
    # f32r: same bits as f32 but streams 1 col/cycle on the PE (vs 4 for
    # plain f32).  The reduce weights are powers of two, so the multiply is
    # exact in any format; accumulation is fp32 PSUM either way.
    f32r = mybir.dt.float32r
    bf16 = mybir.dt.bfloat16
    f16 = mybir.dt.float16
    f8 = mybir.dt.float8e3

    # merged weight stream: per (g,kk) block of _BLK B per partition row =
    # [whi f16 1024 B | wlo8 f8 512 B (when USE_FP8)] — one DMA stream with
    # large contiguous lines (bigger packets -> ~line-rate HBM)
    u8 = mybir.dt.uint8
    wmix_dram = nc.dram_tensor("wmix", [P, n_kk * 4 * _BLK], u8, kind="ExternalInput")
    lhs_dram = nc.dram_tensor("lhs", [P, 2 * n_kk], f16, kind="ExternalInput")
    lhs8_dram = nc.dram_tensor("lhs8", [P, n_kk], f8, kind="ExternalInput")
    bias_dram = nc.dram_tensor("bias", [1, 4096], bf16, kind="ExternalInput")
    one_dram = nc.dram_tensor("one", [1, 1], bf16, kind="ExternalInput")
    red_dram = nc.dram_tensor("redvec", [33, 1], f32r, kind="ExternalInput")
    ct_dram = nc.dram_tensor("ct", [1, HS], f32, kind="ExternalInput")
    out_dram = nc.dram_tensor("h_out", [1, HS], f32, kind="ExternalOutput")

    n_slabs = n_kk // SLABK
    slab_cols = SLABK * _BLK

    with tile.TileContext(nc) as tc:
        with (
            tc.tile_pool(name="const", bufs=1) as const_pool,
            tc.tile_pool(name="wpool", bufs=W_BUFS) as w_pool,
            tc.tile_pool(name="psum", bufs=1, space=bass.MemorySpace.PSUM) as psum_pool,
            tc.tile_pool(name="epi", bufs=1) as epi_pool,
        ):
            # zeros for the group-opening zero-matmuls (DVE memset, no DMA dep)
            wz = const_pool.tile([P, 512], bf16, tag="wz")
            nc.vector.memset(wz[:, :], 0.0)
            psumB = [
                psum_pool.tile([1, HS], f32, tag=f"pb{g}", name=f"psumB{g}")
                for g in range(4)
            ]
            for i in range(N_WARMUP):
                nc.tensor.matmul(
                    psumB[3][0:1, :], wz[:, 0:1], wz[:, :], start=True, stop=True
                )

            # --- constants (ACT ring, ahead of the wlo slabs) ---
            lhs_sb = const_pool.tile([P, 2 * n_kk], f16, tag="lhs")
            lhs8_sb = const_pool.tile([P, n_kk], f8, tag="lhs8")
            bias_sb = const_pool.tile([1, 4096], bf16, tag="bias")
            one_sb = const_pool.tile([1, 1], bf16, tag="one")
            red_sb = const_pool.tile([33, 1], f32r, tag="red")
            ct_sb = const_pool.tile([1, HS], f32, tag="ct")
            nc.scalar.dma_start(out=lhs_sb[:, :], in_=lhs_dram[:, :])
            nc.scalar.dma_start(out=lhs8_sb[:, :], in_=lhs8_dram[:, :])
            nc.scalar.dma_start(out=bias_sb[:, :], in_=bias_dram[:, :])
            nc.scalar.dma_start(out=one_sb[:, :], in_=one_dram[:, :])
            nc.scalar.dma_start(out=red_sb[:, :], in_=red_dram[:, :])
            nc.scalar.dma_start(out=ct_sb[:, :], in_=ct_dram[:, :])

            # [33, 512]: rows 0-1 = M=2 bf16 accum, row 32 = fp8 accum (PSUM
            # base partitions must be 0/32/64), rows 2-31 zeroed and weighted
            # 0 in the reduce.
            psumA = [
                psum_pool.tile([33, HS], f32, tag=f"pa{g}", name=f"psumA{g}")
                for g in range(4)
            ]

            # --- weight stream + matmuls, gate-major ---
            for g in range(4):
                for s in range(n_slabs):
                    col0 = (g * n_kk + s * SLABK) * _BLK
                    wt = w_pool.tile([P, slab_cols], u8, tag="w", name=f"w{g}_{s}")
                    if g == 3 and s == n_slabs - 1:
                        # split the final slab's DMA so the tail matmuls
                        # start as soon as the first half lands (shrinks the
                        # post-DMA pipeline drain)
                        half = slab_cols // 2
                        nc.sync.dma_start(
                            out=wt[:, 0:half], in_=wmix_dram[:, col0:col0 + half]
                        )
                        nc.sync.dma_start(
                            out=wt[:, half:slab_cols],
                            in_=wmix_dram[:, col0 + half:col0 + slab_cols],
                        )
                    else:
                        nc.sync.dma_start(
                            out=wt[:, :], in_=wmix_dram[:, col0:col0 + slab_cols]
                        )
                    for j in range(SLABK):
                        kk = s * SLABK + j
                        first = kk == 0
                        last = kk == n_kk - 1
                        whi_rhs = wt[:, j * _BLK:j * _BLK + 1024].bitcast(f16)
                        if USE_FP8:
                            wlo_rhs = wt[:, j * _BLK + 1024:(j + 1) * _BLK].bitcast(f8)
                        if first:
                            # open the accumulation group: zero rows 0-32
                            nc.tensor.matmul(
                                psumA[g][0:33, :], wz[:, 0:33], wz[:, :],
                                start=True, stop=False,
                            )
                        nc.tensor.matmul(
                            psumA[g][0:2, :],
                            lhs_sb[:, 2 * kk:2 * kk + 2],
                            whi_rhs,
                            start=False,
                            stop=last,
                        )
                        if first:
                            # biases: K=1 matmuls into row 0 (hi + lo)
                            nc.tensor.matmul(
                                psumA[g][0:1, :],
                                one_sb[0:1, 0:1],
                                bias_sb[0:1, (g * 2) * 512:(g * 2 + 1) * 512],
                                start=False, stop=False,
                            )
                            nc.tensor.matmul(
                                psumA[g][0:1, :],
                                one_sb[0:1, 0:1],
                                bias_sb[0:1, (g * 2 + 1) * 512:(g * 2 + 2) * 512],
                                start=False, stop=False,
                            )
                        if USE_FP8:
                            nc.tensor.matmul(
                                psumA[g][32:33, :],
                                lhs8_sb[:, kk:kk + 1],
                                wlo_rhs,
                                start=False,
                                stop=last,
                            )

            # --- per-gate: copy 3 PSUM rows to SBUF, fp32 K=3 reduce matmul
            #     against [1, 1, descale], then the gate activation ---
            act = []
            for g in range(4):
                rows = epi_pool.tile([33, HS], f32r, tag=f"rows{g}", name=f"rows{g}")
                nc.scalar.copy(rows[0:33, :], psumA[g][0:33, :])
                nc.tensor.matmul(
                    psumB[g][0:1, :], red_sb[0:33, 0:1], rows[0:33, :],
                    start=True, stop=True,
                )
                a = epi_pool.tile([1, HS], f32, tag=f"act{g}", name=f"act{g}")
                func = (
                    mybir.ActivationFunctionType.Tanh
                    if g == 2
                    else mybir.ActivationFunctionType.Sigmoid
                )
                nc.scalar.activation(a[0:1, :], psumB[g][0:1, :], func)
                act.append(a)

            i_t, f_t, g_t, o_t = act
            ig = epi_pool.tile([1, HS], f32, tag="ig")
            fc = epi_pool.tile([1, HS], f32, tag="fc")
            cn = epi_pool.tile([1, HS], f32, tag="cn")
            tn = epi_pool.tile([1, HS], f32, tag="tn")
            hh = epi_pool.tile([1, HS], f32, tag="hh")
            nc.vector.tensor_mul(ig[0:1, :], i_t[0:1, :], g_t[0:1, :])
            nc.vector.tensor_mul(fc[0:1, :], f_t[0:1, :], ct_sb[0:1, :])
            nc.vector.tensor_add(cn[0:1, :], ig[0:1, :], fc[0:1, :])
            nc.scalar.activation(tn[0:1, :], cn[0:1, :], mybir.ActivationFunctionType.Tanh)
            nc.vector.tensor_mul(hh[0:1, :], o_t[0:1, :], tn[0:1, :])
            nc.sync.dma_start(out=out_dram[:, :], in_=hh[0:1, :])

    nc.compile()
    return nc


def _split_hi_lo_f32(a: np.ndarray):
    """fp32 -> (bf16-as-f32 hi, f32 residual lo)."""
    a = np.ascontiguousarray(a, dtype=np.float32)
    hi = a.astype(BF16)
    return hi, a - hi.astype(np.float32)


def _split16(a: np.ndarray):
    """fp32 -> (fp16 hi with subnormals flushed to 0, f32 residual lo)."""
    a = np.ascontiguousarray(a, dtype=np.float32)
    hi = a.astype(np.float16)
    hi = np.where(np.abs(hi) < FP16_MIN_NORMAL, np.float16(0), hi)
    return hi, a - hi.astype(np.float32)


def run(inputs: dict, trace: bool = False, trace_cores=None):
    """Returns (h_new [4096] f32, exec_time_ns or None)."""
    if trace:
        _ensure_ntff_hook()
    inputs = {k: np.asarray(v) for k, v in inputs.items()}
    x = inputs["x_t"].astype(np.float32)
    h = inputs["h_t"].astype(np.float32)
    c = inputs["c_t"].astype(np.float32)

    h_zero = not np.any(h)
    n_kk = KX if h_zero else 2 * KX

    if n_kk not in _program_cache:
        _program_cache[n_kk] = _build_program(n_kk)
    nc = _program_cache[n_kk]

    f8max = float(ml_dtypes.finfo(F8).max)

    # lhs vector: x (and h when nonzero), fp16 hi + fp16 lo*2^11 per chunk
    vec = x if h_zero else np.concatenate([x, h]).astype(np.float32)
    vhi, vlo_f = _split16(vec)
    vhi_f = vhi.astype(np.float32)
    vlo = (vlo_f * XL_SHIFT).astype(np.float16)
    vlo = np.where(np.abs(vlo) < FP16_MIN_NORMAL, np.float16(0), vlo)
    lhs = np.ascontiguousarray(
        np.stack(
            [vhi.reshape(n_kk, P), vlo.reshape(n_kk, P)], axis=2
        ).transpose(1, 0, 2).reshape(P, 2 * n_kk)
    )
    # fp8 copy of the hi vector, scaled by 2^b
    vmax = np.abs(vhi_f).max()
    b_exp = float(np.floor(np.log2((f8max / 2) / max(vmax, 1e-30))))
    lhs8 = np.ascontiguousarray(
        (vhi_f * 2.0**b_exp).astype(F8).reshape(n_kk, P).T
    )

    # weight split (full matrices once; slice per core below)
    whis, wlos = [], []
    wlo_max = 0.0
    for g in range(4):
        wx = np.asarray(inputs[_GATES_X[g]], dtype=np.float32)
        if not h_zero:
            wx = np.concatenate(
                [wx, np.asarray(inputs[_GATES_H[g]], dtype=np.float32)], axis=0
            )
        hi, lo_f = _split16(wx)
        wlo_max = max(wlo_max, float(np.abs(lo_f).max()))
        whis.append(hi)
        wlos.append(lo_f)
    a_exp = float(np.floor(np.log2((f8max / 2) / max(wlo_max, 1e-30))))
    descale = np.float32(2.0 ** (-(a_exp + b_exp)))
    redvec = np.zeros((33, 1), dtype=np.float32)
    redvec[0, 0] = 1.0
    redvec[1, 0] = np.float32(1.0 / XL_SHIFT)
    redvec[32, 0] = descale if USE_FP8 else np.float32(0.0)
    one = np.ones((1, 1), dtype=BF16)

    in_maps = []
    for core in range(NCORES):
        sl = slice(core * HS, (core + 1) * HS)
        wmix_blocks = []
        for g in range(4):
            hi = np.ascontiguousarray(whis[g][:, sl])  # [n_kk*128, 512] fp16
            if USE_FP8:
                lo8 = (wlos[g][:, sl] * 2.0**a_exp).astype(F8)
                # per row: [1024 B of fp16 | 512 B of fp8]
                mix = np.concatenate(
                    [hi.view(np.uint8).reshape(n_kk * P, 1024),
                     lo8.view(np.uint8).reshape(n_kk * P, 512)],
                    axis=1,
                )  # [n_kk*128, 1536] u8
            else:
                mix = hi.view(np.uint8).reshape(n_kk * P, 1024)
            wmix_blocks.append(
                mix.reshape(n_kk, P, _BLK).transpose(1, 0, 2).reshape(P, n_kk * _BLK)
            )
        bias = np.empty((1, 4096), dtype=BF16)
        for g in range(4):
            bb = (
                np.asarray(inputs[_BIAS_X[g]], dtype=np.float32)
                + np.asarray(inputs[_BIAS_H[g]], dtype=np.float32)
            )[sl]
            bhi, blo_f = _split_hi_lo_f32(bb)
            bias[0, (g * 2) * 512:(g * 2 + 1) * 512] = bhi
            bias[0, (g * 2 + 1) * 512:(g * 2 + 2) * 512] = blo_f.astype(BF16)
        in_maps.append(
            {
                "wmix": np.ascontiguousarray(np.concatenate(wmix_blocks, axis=1)),
                "lhs": lhs,
                "lhs8": lhs8,
                "bias": bias,
                "one": one,
                "redvec": redvec,
                "ct": np.ascontiguousarray(c[sl]).reshape(1, HS),
            }
        )

    res = run_bass_kernel_spmd(
        nc, in_maps, core_ids=list(range(NCORES)), trace=trace,
        trace_cores=trace_cores,
    )
    if trace_cores and len(trace_cores) > 1:
        print(f"mean exec across cores: {res.mean_exec_time_ns} ns, "
              f"max on core {res.max_exec_time_core_id}: {res.exec_time_ns} ns")
    out = np.concatenate(
        [np.asarray(res.results[core]["h_out"][0], dtype=np.float32)
         for core in range(NCORES)]
    )
    return out, res.exec_time_ns


def _ensure_ntff_hook():
    """Register the axon NTFF profile hook if boot-time registration was
    skipped (antenv.axon_hooks missing from the agent image).  Test-only."""
    import os
    import sys
    import types

    try:
        from antenv.axon_hooks import get_axon_ntff_profile_hook  # noqa: F401
        return
    except ImportError:
        pass
    mod = types.ModuleType("antenv.axon_hooks")
    mod._hook = None

    def set_axon_ntff_profile_hook(h):
        mod._hook = h

    def get_axon_ntff_profile_hook():
        return mod._hook

    mod.set_axon_ntff_profile_hook = set_axon_ntff_profile_hook
    mod.get_axon_ntff_profile_hook = get_axon_ntff_profile_hook
    sys.modules["antenv.axon_hooks"] = mod
    try:
        import antenv

        antenv.axon_hooks = mod
    except ImportError:
        pass
    try:
        from trn_agent_boot.trn_boot import _ntff_profile_via_ctypes

        for so in ("/opt/axon/libaxon_pjrt.so", "/root/.axon_site/libaxon_pjrt.so"):
            if os.path.exists(so):
                mod._hook = _ntff_profile_via_ctypes(so)
                break
    except Exception as e:  # degrade to no-trace
        print(f"ntff hook unavailable: {e!r}", file=sys.stderr)


def kernel(**inputs) -> np.ndarray:
    out, _ = run(inputs)
    return out


# revision 42
# speedup vs baseline: 1.5564x; 1.0983x over previous
"""Single-step LSTM cell (NaiveLayerLSTM, INPUT_SZ=HIDDEN_SZ=4096) on 8 trn2
NeuronCores.

Sharding (tensor-parallel, per the sharding hint): core c owns hidden columns
[c*512, (c+1)*512) of every gate's weight matrix; x_t/h_t are replicated; each
core computes its 512-wide slice of the i/f/g/o gates and the c/h update
locally; the host concatenates the 8 h_new slices.  Single step, so no
collectives.

Numerics / precision scheme (per 128-row contraction chunk kk):
    x = xh + xl/2^11   (fp16 hi + fp16 lo-scaled-by-2^11, split on host)
    W = Whi [+ Wlo]    (fp16 hi [+ fp8e3m4 lo when USE_FP8, prescaled 2^a])
    x@W ~= xh@Whi + 2^-11*(xl@Whi) [+ 2^-(a+b)*(xh*2^b)@(Wlo*2^a)]
with all accumulation in fp32 PSUM:
  - one M=2 fp16 matmul with lhsT=[xh,xl] computes xh@Whi and xl@Whi in a
    single 512-cycle pass (PSUM rows 0/1),
  - (USE_FP8) one fp8 e3m4 matmul accumulates the lo-correction into PSUM
    row 32 (PSUM matmul base partitions must be 0/32/64),
  - biases enter PSUM row 0 via K=1 matmuls against a constant 1.0 (bf16
    hi+lo pair),
  - a tiny fp32r K=33 matmul against [1, 2^-11, 0...0, descale] reduces the
    rows (cross-partition sums are impossible on DVE/ACT, trivial on PE;
    the weights are powers of two so fp32r's reduced multiply is exact).
fp16 values in the subnormal range are flushed to zero on the host (the lo
terms absorb them), so host math matches the PE bit-for-bit regardless of
its subnormal handling; the 2^11 scale on xl keeps xl itself out of the
subnormal range.  Measured end-to-end error vs the fp32 reference:
~3.5e-4 absolute (fp16-only default) / ~9e-6 (USE_FP8) on an output of
scale 0.62.

Why this shape: the kernel is HBM-bound — 16 MiB of weight DMA per core
streams at ~398 GB/s (measured, = per-core HBM share) in one continuous
single-ring stream of 2 MiB slabs with 16 KiB per-partition lines; the PE
consumes each slab behind the DMA (1 pass of N=512 per chunk per gate at
1 col/cycle).  Native fp32 matmuls would run at 1/4 rate and fp32 DMA
would be 32 MiB; the fp16 hi/lo split of x keeps the x-side error at
~2^-22 so the only error is the fp16 quantization of W.

If h_t is all zeros (the module default initial state) the h_t@W_h* half of
the contraction is skipped entirely (checked on the actual data at runtime,
so the kernel stays correct for any input).
"""

import numpy as np
import ml_dtypes

import concourse.bass as bass
import concourse.tile as tile
from concourse import bacc, mybir
from concourse.bass_utils import run_bass_kernel_spmd

BF16 = ml_dtypes.bfloat16
F8 = ml_dtypes.float8_e3m4  # matches mybir.dt.float8e3
FP16_MIN_NORMAL = 2.0 ** -14
XL_SHIFT = 2.0 ** 11
P = 128
H = 4096
NCORES = 8
HS = H // NCORES  # 512 per-core hidden slice
KX = H // P       # 32 contraction chunks for the x half
W_BUFS = 6
# PE warm-up matmuls: only useful when the PE is the critical resource from
# the first real matmul.  In the DMA-bound stream the HAM ramp hides inside
# PE slack, and warm-ups DELAY the real stream (head-of-line on the PE FIFO)
# — measured +7.7 us of PE lag.  Keep 0.
N_WARMUP = 0

# True: +fp8e3m4 lo-correction of the fp16 weights (24 MiB/core DMA, ~9e-6
# abs err, ~88 us).  False: fp16 weights only (16 MiB/core DMA, ~3.5e-4 abs
# err, ~70 us).  Both are far inside bf16-class tolerance (~4e-3); default
# to the faster one.
USE_FP8 = False
# bytes per (gate, chunk) block per partition row in the merged weight stream
_BLK = 1536 if USE_FP8 else 1024
# kk-chunks per weight DMA slab: keep partition lines >= 12 KiB so DMA
# packets stay large (small packets measured ~15% below line rate)
SLABK = 8 if USE_FP8 else 16
# even/odd chunk matmuls go to PSUM partition pairs 0-1 / 32-33 (distinct PE
# column groups -> they execute CONCURRENTLY in the array, halving effective
# PE time and making the PE immune to HAM cold-clock oscillation)
_PAIR = not USE_FP8
_ROWS = 34 if _PAIR else 33

_GATES_X = ["W_ii", "W_if", "W_ig", "W_io"]
_GATES_H = ["W_hi", "W_hf", "W_hg", "W_ho"]
_BIAS_X = ["b_ii", "b_if", "b_ig", "b_io"]
_BIAS_H = ["b_hi", "b_hf", "b_hg", "b_ho"]

_program_cache: dict = {}


def _build_program(n_kk: int):
    nc = bacc.Bacc(
        "TRN2",
        target_bir_lowering=False,
        debug=False,
        enable_asserts=False,
        num_devices=NCORES,
    )
    f32 = mybir.dt.float32
    # f32r: same bits as f32 but streams 1 col/cycle on the PE (vs 4 for
    # plain f32).  The reduce weights are powers of two, so the multiply is
    # exact in any format; accumulation is fp32 PSUM either way.
    f32r = mybir.dt.float32r
    bf16 = mybir.dt.bfloat16
    f16 = mybir.dt.float16
    f8 = mybir.dt.float8e3

    # merged weight stream: per (g,kk) block of _BLK B per partition row =
    # [whi f16 1024 B | wlo8 f8 512 B (when USE_FP8)] — one DMA stream with
    # large contiguous lines (bigger packets -> ~line-rate HBM)
    u8 = mybir.dt.uint8
    wmix_dram = nc.dram_tensor("wmix", [P, n_kk * 4 * _BLK], u8, kind="ExternalInput")
    lhs_dram = nc.dram_tensor("lhs", [P, 2 * n_kk], f16, kind="ExternalInput")
    lhs8_dram = nc.dram_tensor("lhs8", [P, n_kk], f8, kind="ExternalInput")
    bias_dram = nc.dram_tensor("bias", [1, 4096], bf16, kind="ExternalInput")
    one_dram = nc.dram_tensor("one", [1, 1], bf16, kind="ExternalInput")
    red_dram = nc.dram_tensor("redvec", [_ROWS, 1], f32r, kind="ExternalInput")
    ct_dram = nc.dram_tensor("ct", [1, HS], f32, kind="ExternalInput")
    out_dram = nc.dram_tensor("h_out", [1, HS], f32, kind="ExternalOutput")

    n_slabs = n_kk // SLABK
    slab_cols = SLABK * _BLK

    with tile.TileContext(nc) as tc:
        with (
            tc.tile_pool(name="const", bufs=1) as const_pool,
            tc.tile_pool(name="wpool", bufs=W_BUFS) as w_pool,
            tc.tile_pool(name="psum", bufs=1, space=bass.MemorySpace.PSUM) as psum_pool,
            tc.tile_pool(name="epi", bufs=1) as epi_pool,
        ):
            # zeros for the group-opening zero-matmuls (DVE memset, no DMA dep)
            wz = const_pool.tile([P, 512], bf16, tag="wz")
            nc.vector.memset(wz[:, :], 0.0)
            psumB = [
                psum_pool.tile([1, HS], f32, tag=f"pb{g}", name=f"psumB{g}")
                for g in range(4)
            ]
            for i in range(N_WARMUP):
                nc.tensor.matmul(
                    psumB[3][0:1, :], wz[:, 0:1], wz[:, :], start=True, stop=True
                )

            # --- constants (ACT ring, ahead of the wlo slabs) ---
            lhs_sb = const_pool.tile([P, 2 * n_kk], f16, tag="lhs")
            lhs8_sb = const_pool.tile([P, n_kk], f8, tag="lhs8")
            bias_sb = const_pool.tile([1, 4096], bf16, tag="bias")
            one_sb = const_pool.tile([1, 1], bf16, tag="one")
            red_sb = const_pool.tile([_ROWS, 1], f32r, tag="red")
            ct_sb = const_pool.tile([1, HS], f32, tag="ct")
            nc.scalar.dma_start(out=lhs_sb[:, :], in_=lhs_dram[:, :])
            nc.scalar.dma_start(out=lhs8_sb[:, :], in_=lhs8_dram[:, :])
            nc.scalar.dma_start(out=bias_sb[:, :], in_=bias_dram[:, :])
            nc.scalar.dma_start(out=one_sb[:, :], in_=one_dram[:, :])
            nc.scalar.dma_start(out=red_sb[:, :], in_=red_dram[:, :])
            nc.scalar.dma_start(out=ct_sb[:, :], in_=ct_dram[:, :])

            # [33, 512]: rows 0-1 = M=2 bf16 accum, row 32 = fp8 accum (PSUM
            # base partitions must be 0/32/64), rows 2-31 zeroed and weighted
            # 0 in the reduce.
            psumA = [
                psum_pool.tile([_ROWS, HS], f32, tag=f"pa{g}", name=f"psumA{g}")
                for g in range(4)
            ]

            # --- weight stream + matmuls, gate-major ---
            for g in range(4):
                for s in range(n_slabs):
                    col0 = (g * n_kk + s * SLABK) * _BLK
                    wt = w_pool.tile([P, slab_cols], u8, tag="w", name=f"w{g}_{s}")
                    if g == 3 and s == n_slabs - 1:
                        # split the final slab's DMA so the tail matmuls
                        # start as soon as the first half lands (shrinks the
                        # post-DMA pipeline drain)
                        half = slab_cols // 2
                        nc.sync.dma_start(
                            out=wt[:, 0:half], in_=wmix_dram[:, col0:col0 + half]
                        )
                        nc.sync.dma_start(
                            out=wt[:, half:slab_cols],
                            in_=wmix_dram[:, col0 + half:col0 + slab_cols],
                        )
                    else:
                        nc.sync.dma_start(
                            out=wt[:, :], in_=wmix_dram[:, col0:col0 + slab_cols]
                        )
                    for j in range(SLABK):
                        kk = s * SLABK + j
                        first = kk == 0
                        last = kk == n_kk - 1
                        whi_rhs = wt[:, j * _BLK:j * _BLK + 1024].bitcast(f16)
                        if USE_FP8:
                            wlo_rhs = wt[:, j * _BLK + 1024:(j + 1) * _BLK].bitcast(f8)
                        if first:
                            # open the accumulation group: zero all rows
                            nc.tensor.matmul(
                                psumA[g][0:_ROWS, :], wz[:, 0:_ROWS], wz[:, :],
                                start=True, stop=False,
                            )
                        if _PAIR and kk % 2 == 1:
                            out_rows = psumA[g][32:34, :]
                            stop_now = kk == n_kk - 1
                        else:
                            out_rows = psumA[g][0:2, :]
                            stop_now = kk == (n_kk - 2 if _PAIR else n_kk - 1)
                        nc.tensor.matmul(
                            out_rows,
                            lhs_sb[:, 2 * kk:2 * kk + 2],
                            whi_rhs,
                            start=False,
                            stop=stop_now,
                        )
                        if first:
                            # biases: K=1 matmuls into row 0 (hi + lo)
                            nc.tensor.matmul(
                                psumA[g][0:1, :],
                                one_sb[0:1, 0:1],
                                bias_sb[0:1, (g * 2) * 512:(g * 2 + 1) * 512],
                                start=False, stop=False,
                            )
                            nc.tensor.matmul(
                                psumA[g][0:1, :],
                                one_sb[0:1, 0:1],
                                bias_sb[0:1, (g * 2 + 1) * 512:(g * 2 + 2) * 512],
                                start=False, stop=False,
                            )
                        if USE_FP8:
                            nc.tensor.matmul(
                                psumA[g][32:33, :],
                                lhs8_sb[:, kk:kk + 1],
                                wlo_rhs,
                                start=False,
                                stop=last,
                            )

            # --- per-gate: copy 3 PSUM rows to SBUF, fp32 K=3 reduce matmul
            #     against [1, 1, descale], then the gate activation ---
            act = []
            for g in range(4):
                rows = epi_pool.tile([_ROWS, HS], f32r, tag=f"rows{g}", name=f"rows{g}")
                nc.scalar.copy(rows[0:_ROWS, :], psumA[g][0:_ROWS, :])
                nc.tensor.matmul(
                    psumB[g][0:1, :], red_sb[0:_ROWS, 0:1], rows[0:_ROWS, :],
                    start=True, stop=True,
                )
                a = epi_pool.tile([1, HS], f32, tag=f"act{g}", name=f"act{g}")
                func = (
                    mybir.ActivationFunctionType.Tanh
                    if g == 2
                    else mybir.ActivationFunctionType.Sigmoid
                )
                nc.scalar.activation(a[0:1, :], psumB[g][0:1, :], func)
                act.append(a)

            i_t, f_t, g_t, o_t = act
            ig = epi_pool.tile([1, HS], f32, tag="ig")
            fc = epi_pool.tile([1, HS], f32, tag="fc")
            cn = epi_pool.tile([1, HS], f32, tag="cn")
            tn = epi_pool.tile([1, HS], f32, tag="tn")
            hh = epi_pool.tile([1, HS], f32, tag="hh")
            nc.vector.tensor_mul(ig[0:1, :], i_t[0:1, :], g_t[0:1, :])
            nc.vector.tensor_mul(fc[0:1, :], f_t[0:1, :], ct_sb[0:1, :])
            nc.vector.tensor_add(cn[0:1, :], ig[0:1, :], fc[0:1, :])
            nc.scalar.activation(tn[0:1, :], cn[0:1, :], mybir.ActivationFunctionType.Tanh)
            nc.vector.tensor_mul(hh[0:1, :], o_t[0:1, :], tn[0:1, :])
            nc.sync.dma_start(out=out_dram[:, :], in_=hh[0:1, :])

    nc.compile()
    return nc


def _split_hi_lo_f32(a: np.ndarray):
    """fp32 -> (bf16-as-f32 hi, f32 residual lo)."""
    a = np.ascontiguousarray(a, dtype=np.float32)
    hi = a.astype(BF16)
    return hi, a - hi.astype(np.float32)


def _split16(a: np.ndarray):
    """fp32 -> (fp16 hi with subnormals flushed to 0, f32 residual lo)."""
    a = np.ascontiguousarray(a, dtype=np.float32)
    hi = a.astype(np.float16)
    hi = np.where(np.abs(hi) < FP16_MIN_NORMAL, np.float16(0), hi)
    return hi, a - hi.astype(np.float32)


def run(inputs: dict, trace: bool = False, trace_cores=None):
    """Returns (h_new [4096] f32, exec_time_ns or None)."""
    if trace:
        _ensure_ntff_hook()
    inputs = {k: np.asarray(v) for k, v in inputs.items()}
    x = inputs["x_t"].astype(np.float32)
    h = inputs["h_t"].astype(np.float32)
    c = inputs["c_t"].astype(np.float32)

    h_zero = not np.any(h)
    n_kk = KX if h_zero else 2 * KX

    if n_kk not in _program_cache:
        _program_cache[n_kk] = _build_program(n_kk)
    nc = _program_cache[n_kk]

    f8max = float(ml_dtypes.finfo(F8).max)

    # lhs vector: x (and h when nonzero), fp16 hi + fp16 lo*2^11 per chunk
    vec = x if h_zero else np.concatenate([x, h]).astype(np.float32)
    vhi, vlo_f = _split16(vec)
    vhi_f = vhi.astype(np.float32)
    vlo = (vlo_f * XL_SHIFT).astype(np.float16)
    vlo = np.where(np.abs(vlo) < FP16_MIN_NORMAL, np.float16(0), vlo)
    lhs = np.ascontiguousarray(
        np.stack(
            [vhi.reshape(n_kk, P), vlo.reshape(n_kk, P)], axis=2
        ).transpose(1, 0, 2).reshape(P, 2 * n_kk)
    )
    # fp8 copy of the hi vector, scaled by 2^b
    vmax = np.abs(vhi_f).max()
    b_exp = float(np.floor(np.log2((f8max / 2) / max(vmax, 1e-30))))
    lhs8 = np.ascontiguousarray(
        (vhi_f * 2.0**b_exp).astype(F8).reshape(n_kk, P).T
    )

    # weight split (full matrices once; slice per core below)
    whis, wlos = [], []
    wlo_max = 0.0
    for g in range(4):
        wx = np.asarray(inputs[_GATES_X[g]], dtype=np.float32)
        if not h_zero:
            wx = np.concatenate(
                [wx, np.asarray(inputs[_GATES_H[g]], dtype=np.float32)], axis=0
            )
        hi, lo_f = _split16(wx)
        wlo_max = max(wlo_max, float(np.abs(lo_f).max()))
        whis.append(hi)
        wlos.append(lo_f)
    a_exp = float(np.floor(np.log2((f8max / 2) / max(wlo_max, 1e-30))))
    descale = np.float32(2.0 ** (-(a_exp + b_exp)))
    redvec = np.zeros((_ROWS, 1), dtype=np.float32)
    redvec[0, 0] = 1.0
    redvec[1, 0] = np.float32(1.0 / XL_SHIFT)
    if USE_FP8:
        redvec[32, 0] = descale
    if _PAIR:
        redvec[32, 0] = 1.0
        redvec[33, 0] = np.float32(1.0 / XL_SHIFT)
    one = np.ones((1, 1), dtype=BF16)

    in_maps = []
    for core in range(NCORES):
        sl = slice(core * HS, (core + 1) * HS)
        wmix_blocks = []
        for g in range(4):
            hi = np.ascontiguousarray(whis[g][:, sl])  # [n_kk*128, 512] fp16
            if USE_FP8:
                lo8 = (wlos[g][:, sl] * 2.0**a_exp).astype(F8)
                # per row: [1024 B of fp16 | 512 B of fp8]
                mix = np.concatenate(
                    [hi.view(np.uint8).reshape(n_kk * P, 1024),
                     lo8.view(np.uint8).reshape(n_kk * P, 512)],
                    axis=1,
                )  # [n_kk*128, 1536] u8
            else:
                mix = hi.view(np.uint8).reshape(n_kk * P, 1024)
            wmix_blocks.append(
                mix.reshape(n_kk, P, _BLK).transpose(1, 0, 2).reshape(P, n_kk * _BLK)
            )
        bias = np.empty((1, 4096), dtype=BF16)
        for g in range(4):
            bb = (
                np.asarray(inputs[_BIAS_X[g]], dtype=np.float32)
                + np.asarray(inputs[_BIAS_H[g]], dtype=np.float32)
            )[sl]
            bhi, blo_f = _split_hi_lo_f32(bb)
            bias[0, (g * 2) * 512:(g * 2 + 1) * 512] = bhi
            bias[0, (g * 2 + 1) * 512:(g * 2 + 2) * 512] = blo_f.astype(BF16)
        in_maps.append(
            {
                "wmix": np.ascontiguousarray(np.concatenate(wmix_blocks, axis=1)),
                "lhs": lhs,
                "lhs8": lhs8,
                "bias": bias,
                "one": one,
                "redvec": redvec,
                "ct": np.ascontiguousarray(c[sl]).reshape(1, HS),
            }
        )

    res = run_bass_kernel_spmd(
        nc, in_maps, core_ids=list(range(NCORES)), trace=trace,
        trace_cores=trace_cores,
    )
    if trace_cores and len(trace_cores) > 1:
        print(f"mean exec across cores: {res.mean_exec_time_ns} ns, "
              f"max on core {res.max_exec_time_core_id}: {res.exec_time_ns} ns")
    out = np.concatenate(
        [np.asarray(res.results[core]["h_out"][0], dtype=np.float32)
         for core in range(NCORES)]
    )
    return out, res.exec_time_ns


def _ensure_ntff_hook():
    """Register the axon NTFF profile hook if boot-time registration was
    skipped (antenv.axon_hooks missing from the agent image).  Test-only."""
    import os
    import sys
    import types

    try:
        from antenv.axon_hooks import get_axon_ntff_profile_hook  # noqa: F401
        return
    except ImportError:
        pass
    mod = types.ModuleType("antenv.axon_hooks")
    mod._hook = None

    def set_axon_ntff_profile_hook(h):
        mod._hook = h

    def get_axon_ntff_profile_hook():
        return mod._hook

    mod.set_axon_ntff_profile_hook = set_axon_ntff_profile_hook
    mod.get_axon_ntff_profile_hook = get_axon_ntff_profile_hook
    sys.modules["antenv.axon_hooks"] = mod
    try:
        import antenv

        antenv.axon_hooks = mod
    except ImportError:
        pass
    try:
        from trn_agent_boot.trn_boot import _ntff_profile_via_ctypes

        for so in ("/opt/axon/libaxon_pjrt.so", "/root/.axon_site/libaxon_pjrt.so"):
            if os.path.exists(so):
                mod._hook = _ntff_profile_via_ctypes(so)
                break
    except Exception as e:  # degrade to no-trace
        print(f"ntff hook unavailable: {e!r}", file=sys.stderr)


def kernel(**inputs) -> np.ndarray:
    out, _ = run(inputs)
    return out


# revision 44
# speedup vs baseline: 1.7486x; 1.1235x over previous
"""Single-step LSTM cell (NaiveLayerLSTM, INPUT_SZ=HIDDEN_SZ=4096) on 8 trn2
NeuronCores.

Sharding (tensor-parallel, per the sharding hint): core c owns hidden columns
[c*512, (c+1)*512) of every gate's weight matrix; x_t/h_t are replicated; each
core computes its 512-wide slice of the i/f/g/o gates and the c/h update
locally; the host concatenates the 8 h_new slices.  Single step, so no
collectives.

Numerics / precision scheme (per 128-row contraction chunk kk):
    x = xh + xl/2^11   (fp16 hi + fp16 lo-scaled-by-2^11, split on host)
    W = Whi [+ Wlo]    (fp16 hi [+ fp8e3m4 lo when USE_FP8, prescaled 2^a])
    x@W ~= xh@Whi + 2^-11*(xl@Whi) [+ 2^-(a+b)*(xh*2^b)@(Wlo*2^a)]
with all accumulation in fp32 PSUM:
  - one M=2 fp16 matmul with lhsT=[xh,xl] computes xh@Whi and xl@Whi in a
    single 512-cycle pass (PSUM rows 0/1),
  - (USE_FP8) one fp8 e3m4 matmul accumulates the lo-correction into PSUM
    row 32 (PSUM matmul base partitions must be 0/32/64),
  - biases enter PSUM row 0 via K=1 matmuls against a constant 1.0 (bf16
    hi+lo pair),
  - a tiny fp32r K=33 matmul against [1, 2^-11, 0...0, descale] reduces the
    rows (cross-partition sums are impossible on DVE/ACT, trivial on PE;
    the weights are powers of two so fp32r's reduced multiply is exact).
fp16 values in the subnormal range are flushed to zero on the host (the lo
terms absorb them), so host math matches the PE bit-for-bit regardless of
its subnormal handling; the 2^11 scale on xl keeps xl itself out of the
subnormal range.  Measured end-to-end error vs the fp32 reference:
~3.5e-4 absolute (fp16-only default) / ~9e-6 (USE_FP8) on an output of
scale 0.62.

Why this shape: the kernel is HBM-bound — 16 MiB of weight DMA per core
streams at ~398 GB/s (measured, = per-core HBM share) in one continuous
single-ring stream of 2 MiB slabs with 16 KiB per-partition lines; the PE
consumes each slab behind the DMA (1 pass of N=512 per chunk per gate at
1 col/cycle).  Native fp32 matmuls would run at 1/4 rate and fp32 DMA
would be 32 MiB; the fp16 hi/lo split of x keeps the x-side error at
~2^-22 so the only error is the fp16 quantization of W.

If h_t is all zeros (the module default initial state) the h_t@W_h* half of
the contraction is skipped entirely (checked on the actual data at runtime,
so the kernel stays correct for any input).
"""

import numpy as np
import ml_dtypes

import concourse.bass as bass
import concourse.tile as tile
from concourse import bacc, mybir
from concourse.bass_utils import run_bass_kernel_spmd

BF16 = ml_dtypes.bfloat16
F8 = ml_dtypes.float8_e3m4  # matches mybir.dt.float8e3
FP16_MIN_NORMAL = 2.0 ** -14
XL_SHIFT = 2.0 ** 11
P = 128
H = 4096
NCORES = 8
HS = H // NCORES  # 512 per-core hidden slice
KX = H // P       # 32 contraction chunks for the x half
W_BUFS = 6
# PE warm-up matmuls: only useful when the PE is the critical resource from
# the first real matmul.  In the DMA-bound stream the HAM ramp hides inside
# PE slack, and warm-ups DELAY the real stream (head-of-line on the PE FIFO)
# — measured +7.7 us of PE lag.  Keep 0.
N_WARMUP = 0

# True: +fp8e3m4 lo-correction of the fp16 weights (24 MiB/core DMA, ~9e-6
# abs err, ~88 us).  False: fp16 weights only (16 MiB/core DMA, ~3.5e-4 abs
# err, ~70 us).  Both are far inside bf16-class tolerance (~4e-3); default
# to the faster one.
USE_FP8 = False
# bytes per (gate, chunk) block per partition row in the merged weight stream
_BLK = 1536 if USE_FP8 else 1024
# kk-chunks per weight DMA slab: keep partition lines >= 12 KiB so DMA
# packets stay large (small packets measured ~15% below line rate)
SLABK = 8 if USE_FP8 else 16
# even/odd chunk matmuls go to PSUM partition pairs 0-1 / 32-33 (distinct PE
# column groups -> they execute CONCURRENTLY in the array, halving effective
# PE time and making the PE immune to HAM cold-clock oscillation)
_PAIR = not USE_FP8
_ROWS = 34 if _PAIR else 33

_GATES_X = ["W_ii", "W_if", "W_ig", "W_io"]
_GATES_H = ["W_hi", "W_hf", "W_hg", "W_ho"]
_BIAS_X = ["b_ii", "b_if", "b_ig", "b_io"]
_BIAS_H = ["b_hi", "b_hf", "b_hg", "b_ho"]

_program_cache: dict = {}


def _build_program(n_kk: int, n_g: int = 4):
    # n_g=3: c_t is all zeros -> f_t*c_t == 0 exactly, so the whole W_if
    # matrix is skipped (gates i, g, o only) and c_new = i_t*g_t.
    nc = bacc.Bacc(
        "TRN2",
        target_bir_lowering=False,
        debug=False,
        enable_asserts=False,
        num_devices=NCORES,
    )
    f32 = mybir.dt.float32
    # f32r: same bits as f32 but streams 1 col/cycle on the PE (vs 4 for
    # plain f32).  The reduce weights are powers of two, so the multiply is
    # exact in any format; accumulation is fp32 PSUM either way.
    f32r = mybir.dt.float32r
    bf16 = mybir.dt.bfloat16
    f16 = mybir.dt.float16
    f8 = mybir.dt.float8e3

    # merged weight stream: per (g,kk) block of _BLK B per partition row =
    # [whi f16 1024 B | wlo8 f8 512 B (when USE_FP8)] — one DMA stream with
    # large contiguous lines (bigger packets -> ~line-rate HBM)
    u8 = mybir.dt.uint8
    wmix_dram = nc.dram_tensor("wmix", [P, n_kk * n_g * _BLK], u8, kind="ExternalInput")
    lhs_dram = nc.dram_tensor("lhs", [P, 2 * n_kk], f16, kind="ExternalInput")
    lhs8_dram = nc.dram_tensor("lhs8", [P, n_kk], f8, kind="ExternalInput")
    bias_dram = nc.dram_tensor("bias", [1, n_g * 1024], bf16, kind="ExternalInput")
    one_dram = nc.dram_tensor("one", [1, 1], bf16, kind="ExternalInput")
    red_dram = nc.dram_tensor("redvec", [_ROWS, 1], f32r, kind="ExternalInput")
    ct_dram = nc.dram_tensor("ct", [1, HS], f32, kind="ExternalInput")
    out_dram = nc.dram_tensor("h_out", [1, HS], f32, kind="ExternalOutput")

    n_slabs = n_kk // SLABK
    slab_cols = SLABK * _BLK

    with tile.TileContext(nc) as tc:
        with (
            tc.tile_pool(name="const", bufs=1) as const_pool,
            tc.tile_pool(name="wpool", bufs=W_BUFS) as w_pool,
            tc.tile_pool(name="psum", bufs=1, space=bass.MemorySpace.PSUM) as psum_pool,
            tc.tile_pool(name="epi", bufs=1) as epi_pool,
        ):
            # zeros for the group-opening zero-matmuls (DVE memset, no DMA dep)
            wz = const_pool.tile([P, 512], bf16, tag="wz")
            nc.vector.memset(wz[:, :], 0.0)
            psumB = [
                psum_pool.tile([1, HS], f32, tag=f"pb{g}", name=f"psumB{g}")
                for g in range(n_g)
            ]
            for i in range(N_WARMUP):
                nc.tensor.matmul(
                    psumB[-1][0:1, :], wz[:, 0:1], wz[:, :], start=True, stop=True
                )

            # --- constants (ACT ring, ahead of the wlo slabs) ---
            lhs_sb = const_pool.tile([P, 2 * n_kk], f16, tag="lhs")
            lhs8_sb = const_pool.tile([P, n_kk], f8, tag="lhs8")
            bias_sb = const_pool.tile([1, n_g * 1024], bf16, tag="bias")
            one_sb = const_pool.tile([1, 1], bf16, tag="one")
            red_sb = const_pool.tile([_ROWS, 1], f32r, tag="red")
            ct_sb = const_pool.tile([1, HS], f32, tag="ct")
            nc.scalar.dma_start(out=lhs_sb[:, :], in_=lhs_dram[:, :])
            nc.scalar.dma_start(out=lhs8_sb[:, :], in_=lhs8_dram[:, :])
            nc.scalar.dma_start(out=bias_sb[:, :], in_=bias_dram[:, :])
            nc.scalar.dma_start(out=one_sb[:, :], in_=one_dram[:, :])
            nc.scalar.dma_start(out=red_sb[:, :], in_=red_dram[:, :])
            nc.scalar.dma_start(out=ct_sb[:, :], in_=ct_dram[:, :])

            # [33, 512]: rows 0-1 = M=2 bf16 accum, row 32 = fp8 accum (PSUM
            # base partitions must be 0/32/64), rows 2-31 zeroed and weighted
            # 0 in the reduce.
            psumA = [
                psum_pool.tile([_ROWS, HS], f32, tag=f"pa{g}", name=f"psumA{g}")
                for g in range(n_g)
            ]

            # --- weight stream + matmuls, gate-major ---
            for g in range(n_g):
                for s in range(n_slabs):
                    col0 = (g * n_kk + s * SLABK) * _BLK
                    wt = w_pool.tile([P, slab_cols], u8, tag="w", name=f"w{g}_{s}")
                    if g == n_g - 1 and s == n_slabs - 1:
                        # split the final slab's DMA so the tail matmuls
                        # start as soon as the first half lands (shrinks the
                        # post-DMA pipeline drain)
                        half = slab_cols // 2
                        nc.sync.dma_start(
                            out=wt[:, 0:half], in_=wmix_dram[:, col0:col0 + half]
                        )
                        nc.sync.dma_start(
                            out=wt[:, half:slab_cols],
                            in_=wmix_dram[:, col0 + half:col0 + slab_cols],
                        )
                    else:
                        nc.sync.dma_start(
                            out=wt[:, :], in_=wmix_dram[:, col0:col0 + slab_cols]
                        )
                    for j in range(SLABK):
                        kk = s * SLABK + j
                        first = kk == 0
                        last = kk == n_kk - 1
                        whi_rhs = wt[:, j * _BLK:j * _BLK + 1024].bitcast(f16)
                        if USE_FP8:
                            wlo_rhs = wt[:, j * _BLK + 1024:(j + 1) * _BLK].bitcast(f8)
                        if first:
                            # open the accumulation group: zero all rows
                            nc.tensor.matmul(
                                psumA[g][0:_ROWS, :], wz[:, 0:_ROWS], wz[:, :],
                                start=True, stop=False,
                            )
                        if _PAIR and kk % 2 == 1:
                            out_rows = psumA[g][32:34, :]
                            stop_now = kk == n_kk - 1
                        else:
                            out_rows = psumA[g][0:2, :]
                            stop_now = kk == (n_kk - 2 if _PAIR else n_kk - 1)
                        nc.tensor.matmul(
                            out_rows,
                            lhs_sb[:, 2 * kk:2 * kk + 2],
                            whi_rhs,
                            start=False,
                            stop=stop_now,
                        )
                        if first:
                            # biases: K=1 matmuls into row 0 (hi + lo)
                            nc.tensor.matmul(
                                psumA[g][0:1, :],
                                one_sb[0:1, 0:1],
                                bias_sb[0:1, (g * 2) * 512:(g * 2 + 1) * 512],
                                start=False, stop=False,
                            )
                            nc.tensor.matmul(
                                psumA[g][0:1, :],
                                one_sb[0:1, 0:1],
                                bias_sb[0:1, (g * 2 + 1) * 512:(g * 2 + 2) * 512],
                                start=False, stop=False,
                            )
                        if USE_FP8:
                            nc.tensor.matmul(
                                psumA[g][32:33, :],
                                lhs8_sb[:, kk:kk + 1],
                                wlo_rhs,
                                start=False,
                                stop=last,
                            )

            # --- per-gate: copy 3 PSUM rows to SBUF, fp32 K=3 reduce matmul
            #     against [1, 1, descale], then the gate activation ---
            act = []
            tanh_gate = 2 if n_g == 4 else 1
            for g in range(n_g):
                rows = epi_pool.tile([_ROWS, HS], f32r, tag=f"rows{g}", name=f"rows{g}")
                nc.scalar.copy(rows[0:_ROWS, :], psumA[g][0:_ROWS, :])
                nc.tensor.matmul(
                    psumB[g][0:1, :], red_sb[0:_ROWS, 0:1], rows[0:_ROWS, :],
                    start=True, stop=True,
                )
                a = epi_pool.tile([1, HS], f32, tag=f"act{g}", name=f"act{g}")
                func = (
                    mybir.ActivationFunctionType.Tanh
                    if g == tanh_gate
                    else mybir.ActivationFunctionType.Sigmoid
                )
                nc.scalar.activation(a[0:1, :], psumB[g][0:1, :], func)
                act.append(a)

            ig = epi_pool.tile([1, HS], f32, tag="ig")
            tn = epi_pool.tile([1, HS], f32, tag="tn")
            hh = epi_pool.tile([1, HS], f32, tag="hh")
            if n_g == 4:
                i_t, f_t, g_t, o_t = act
                fc = epi_pool.tile([1, HS], f32, tag="fc")
                cn = epi_pool.tile([1, HS], f32, tag="cn")
                nc.vector.tensor_mul(ig[0:1, :], i_t[0:1, :], g_t[0:1, :])
                nc.vector.tensor_mul(fc[0:1, :], f_t[0:1, :], ct_sb[0:1, :])
                nc.vector.tensor_add(cn[0:1, :], ig[0:1, :], fc[0:1, :])
                nc.scalar.activation(tn[0:1, :], cn[0:1, :], mybir.ActivationFunctionType.Tanh)
            else:
                # c_t == 0: c_new = i_t * g_t
                i_t, g_t, o_t = act
                nc.vector.tensor_mul(ig[0:1, :], i_t[0:1, :], g_t[0:1, :])
                nc.scalar.activation(tn[0:1, :], ig[0:1, :], mybir.ActivationFunctionType.Tanh)
            nc.vector.tensor_mul(hh[0:1, :], o_t[0:1, :], tn[0:1, :])
            nc.sync.dma_start(out=out_dram[:, :], in_=hh[0:1, :])

    nc.compile()
    return nc


def _split_hi_lo_f32(a: np.ndarray):
    """fp32 -> (bf16-as-f32 hi, f32 residual lo)."""
    a = np.ascontiguousarray(a, dtype=np.float32)
    hi = a.astype(BF16)
    return hi, a - hi.astype(np.float32)


def _split16(a: np.ndarray):
    """fp32 -> (fp16 hi with subnormals flushed to 0, f32 residual lo)."""
    a = np.ascontiguousarray(a, dtype=np.float32)
    hi = a.astype(np.float16)
    hi = np.where(np.abs(hi) < FP16_MIN_NORMAL, np.float16(0), hi)
    return hi, a - hi.astype(np.float32)


def run(inputs: dict, trace: bool = False, trace_cores=None):
    """Returns (h_new [4096] f32, exec_time_ns or None)."""
    if trace:
        _ensure_ntff_hook()
    inputs = {k: np.asarray(v) for k, v in inputs.items()}
    x = inputs["x_t"].astype(np.float32)
    h = inputs["h_t"].astype(np.float32)
    c = inputs["c_t"].astype(np.float32)

    h_zero = not np.any(h)
    n_kk = KX if h_zero else 2 * KX
    # c_t == 0 -> f_t * c_t == 0 exactly: skip the forget gate entirely
    c_zero = not np.any(c)
    active = [0, 2, 3] if c_zero else [0, 1, 2, 3]
    n_g = len(active)

    if (n_kk, n_g) not in _program_cache:
        _program_cache[(n_kk, n_g)] = _build_program(n_kk, n_g)
    nc = _program_cache[(n_kk, n_g)]

    f8max = float(ml_dtypes.finfo(F8).max)

    # lhs vector: x (and h when nonzero), fp16 hi + fp16 lo*2^11 per chunk
    vec = x if h_zero else np.concatenate([x, h]).astype(np.float32)
    vhi, vlo_f = _split16(vec)
    vhi_f = vhi.astype(np.float32)
    vlo = (vlo_f * XL_SHIFT).astype(np.float16)
    vlo = np.where(np.abs(vlo) < FP16_MIN_NORMAL, np.float16(0), vlo)
    lhs = np.ascontiguousarray(
        np.stack(
            [vhi.reshape(n_kk, P), vlo.reshape(n_kk, P)], axis=2
        ).transpose(1, 0, 2).reshape(P, 2 * n_kk)
    )
    # fp8 copy of the hi vector, scaled by 2^b
    vmax = np.abs(vhi_f).max()
    b_exp = float(np.floor(np.log2((f8max / 2) / max(vmax, 1e-30))))
    lhs8 = np.ascontiguousarray(
        (vhi_f * 2.0**b_exp).astype(F8).reshape(n_kk, P).T
    )

    # weight split (full matrices once; slice per core below)
    whis, wlos = [], []
    wlo_max = 0.0
    for g in active:
        wx = np.asarray(inputs[_GATES_X[g]], dtype=np.float32)
        if not h_zero:
            wx = np.concatenate(
                [wx, np.asarray(inputs[_GATES_H[g]], dtype=np.float32)], axis=0
            )
        hi, lo_f = _split16(wx)
        wlo_max = max(wlo_max, float(np.abs(lo_f).max()))
        whis.append(hi)
        wlos.append(lo_f)
    a_exp = float(np.floor(np.log2((f8max / 2) / max(wlo_max, 1e-30))))
    descale = np.float32(2.0 ** (-(a_exp + b_exp)))
    redvec = np.zeros((_ROWS, 1), dtype=np.float32)
    redvec[0, 0] = 1.0
    redvec[1, 0] = np.float32(1.0 / XL_SHIFT)
    if USE_FP8:
        redvec[32, 0] = descale
    if _PAIR:
        redvec[32, 0] = 1.0
        redvec[33, 0] = np.float32(1.0 / XL_SHIFT)
    one = np.ones((1, 1), dtype=BF16)

    in_maps = []
    for core in range(NCORES):
        sl = slice(core * HS, (core + 1) * HS)
        wmix_blocks = []
        for gi in range(n_g):
            hi = np.ascontiguousarray(whis[gi][:, sl])  # [n_kk*128, 512] fp16
            if USE_FP8:
                lo8 = (wlos[gi][:, sl] * 2.0**a_exp).astype(F8)
                # per row: [1024 B of fp16 | 512 B of fp8]
                mix = np.concatenate(
                    [hi.view(np.uint8).reshape(n_kk * P, 1024),
                     lo8.view(np.uint8).reshape(n_kk * P, 512)],
                    axis=1,
                )  # [n_kk*128, 1536] u8
            else:
                mix = hi.view(np.uint8).reshape(n_kk * P, 1024)
            wmix_blocks.append(
                mix.reshape(n_kk, P, _BLK).transpose(1, 0, 2).reshape(P, n_kk * _BLK)
            )
        bias = np.empty((1, n_g * 1024), dtype=BF16)
        for gi, g in enumerate(active):
            bb = (
                np.asarray(inputs[_BIAS_X[g]], dtype=np.float32)
                + np.asarray(inputs[_BIAS_H[g]], dtype=np.float32)
            )[sl]
            bhi, blo_f = _split_hi_lo_f32(bb)
            bias[0, (gi * 2) * 512:(gi * 2 + 1) * 512] = bhi
            bias[0, (gi * 2 + 1) * 512:(gi * 2 + 2) * 512] = blo_f.astype(BF16)
        in_maps.append(
            {
                "wmix": np.ascontiguousarray(np.concatenate(wmix_blocks, axis=1)),
                "lhs": lhs,
                "lhs8": lhs8,
                "bias": bias,
                "one": one,
                "redvec": redvec,
                "ct": np.ascontiguousarray(c[sl]).reshape(1, HS),
            }
        )

    res = run_bass_kernel_spmd(
        nc, in_maps, core_ids=list(range(NCORES)), trace=trace,
        trace_cores=trace_cores,
    )
    if trace_cores and len(trace_cores) > 1:
        print(f"mean exec across cores: {res.mean_exec_time_ns} ns, "
              f"max on core {res.max_exec_time_core_id}: {res.exec_time_ns} ns")
    out = np.concatenate(
        [np.asarray(res.results[core]["h_out"][0], dtype=np.float32)
         for core in range(NCORES)]
    )
    return out, res.exec_time_ns


def _ensure_ntff_hook():
    """Register the axon NTFF profile hook if boot-time registration was
    skipped (antenv.axon_hooks missing from the agent image).  Test-only."""
    import os
    import sys
    import types

    try:
        from antenv.axon_hooks import get_axon_ntff_profile_hook  # noqa: F401
        return
    except ImportError:
        pass
    mod = types.ModuleType("antenv.axon_hooks")
    mod._hook = None

    def set_axon_ntff_profile_hook(h):
        mod._hook = h

    def get_axon_ntff_profile_hook():
        return mod._hook

    mod.set_axon_ntff_profile_hook = set_axon_ntff_profile_hook
    mod.get_axon_ntff_profile_hook = get_axon_ntff_profile_hook
    sys.modules["antenv.axon_hooks"] = mod
    try:
        import antenv

        antenv.axon_hooks = mod
    except ImportError:
        pass
    try:
        from trn_agent_boot.trn_boot import _ntff_profile_via_ctypes

        for so in ("/opt/axon/libaxon_pjrt.so", "/root/.axon_site/libaxon_pjrt.so"):
            if os.path.exists(so):
                mod._hook = _ntff_profile_via_ctypes(so)
                break
    except Exception as e:  # degrade to no-trace
        print(f"ntff hook unavailable: {e!r}", file=sys.stderr)


def kernel(**inputs) -> np.ndarray:
    out, _ = run(inputs)
    return out
